# revision 52
# baseline (speedup 1.0000x reference)
"""Trainium2 Bass kernel for grouped vector attention (sparse_attention).

Reference computation (B=2, L1=L2=512, D=256, g=16, n=16):
    Q = x_target @ Wq.T ; K = x_source @ Wk.T ; V = x_source @ Wv.T
    diff = Q.reshape(B,L1,1,n,g) - K.reshape(B,1,L2,n,g)
    scores = relu(einsum('bijng,g->bijn', relu(diff), w_mlp) + b_mlp)
    att = softmax(scores, axis=2)                      # over L2
    out = einsum('bijn,bjgn->bign', att, V.reshape(B,L2,g,n)).reshape(B,L1,D)

Sharding: 8 cores = 2 batches x 4 L2(j)-quarters. Each core handles all 512
queries against its 128 source positions and produces partial (unnormalized)
outputs + partial softmax denominators; the host sums the 4 partials per
batch and divides. Sharding over j (not i) means the exp'd scores come out
with j on partitions — exactly what the att@V contraction needs, so there is
no on-chip transpose anywhere.

Per-core pipeline, per source position j (128 of them):
  - t[d,(h,i)] = relu(8Q[i,d] - 8K[j,d]) with d on partitions (Wq/Wk are
    pre-scaled x8 on host). Each j is assigned to ONE elementwise engine:
      ScalarE j's  -> t in fp8e4m3 (ACT pays no fp8 penalty), scores via ONE
                      fp8 DoubleRow matmul (contracts all 256 d at once,
                      128-wide sel variant per jj slot since walrus rejects
                      DoubleRow + tile_position col offsets)
      VectorE j's  -> t in bf16 (DVE 2x mode; fp8-out measured +105ns), via
                      two bf16 matmuls with 32-col sel at tile_position.
    Scores land x16 scaled in the quad's 32-row PSUM slot.
  - per PAIR of quads: one exp over the [128,1024] 2-bank PSUM span
    (p = exp(z/16 + b)), one pc = max(p,1) on DVE, one paired denominator
    matmul (ones_sel.T @ pc_pair -> [16,1024], host sums the halves).
  - V path bf16: V_sel[g][32*jj+nn, e] = V[4g+jj, e]*(e%16==nn) built per
    quad-pair by broadcast-DMA from a DRAM copy of V + masked mult;
    out_partial[e,i] += V_sel.T @ pc  (PSUM accumulation over all quads).
  - exp/pc/V-matmul issue is software-pipelined one pair behind the
    elementwise+score stream so no engine queue head-of-line blocks.
"""

import numpy as np

import concourse.bass as bass
import concourse.bacc as bacc
import concourse.tile as tile
import concourse.mybir as mybir
from concourse.bass_utils import run_bass_kernel_spmd

import ml_dtypes

F32 = mybir.dt.float32
BF16 = mybir.dt.bfloat16
FP8 = mybir.dt.float8e4
AL = mybir.AluOpType
AF = mybir.ActivationFunctionType
DR = mybir.MatmulPerfMode.DoubleRow

B, L1, L2, D = 2, 512, 512, 256
G = 16           # group size (d_group)
N = 16           # number of groups
NCORES = 8
JSH = 128        # source positions per core (L2 / 4)
NQUAD = 32       # 32 quads of 4 source positions
NPAIR = 16       # pairs of quads
BF = ml_dtypes.bfloat16
F8 = ml_dtypes.float8_e4m3

SCALE_T = 8.0    # folded into Wq/Wk on host: t = relu(8q-8k) in e4m3 range
SEL_SCALE = 2.0  # fp8 sel = 2*w so fp8 weights stay in normal range
EXP_SCALE = 1.0 / (SCALE_T * SEL_SCALE)


def _s_jjs(g):
    """Which jj slots of quad g go to the ScalarE elementwise path.
    2.5 S-j's per pair (5 ACT units) balances ScalarE (707ns/unit + the
    paired exp) against VectorE (345ns/unit + pc + V_sel masking). S-path
    work is issued LAST in each quad so the PE never waits on ScalarE."""
    return (2, 3) if g % 8 == 7 else (3,)


def _build(b_val: float):
    """Build + compile the per-core Bass graph. Same graph for all 8 cores."""
    nc = bacc.Bacc(
        "TRN2", target_bir_lowering=False, debug=False, enable_asserts=False
    )

    # ---- DRAM parameters (per-core shards, host-prepped) ----
    xtT_d = nc.dram_tensor("xtT", [128, 2 * L1], BF16, kind="ExternalInput")
    xssT_d = nc.dram_tensor("xssT", [128, 2 * JSH], BF16, kind="ExternalInput")
    wqT_d = nc.dram_tensor("wqT", [128, 2 * D], BF16, kind="ExternalInput")
    wkT_d = nc.dram_tensor("wkT", [128, 2 * D], BF16, kind="ExternalInput")
    wvT_d = nc.dram_tensor("wvT", [128, 2 * D], BF16, kind="ExternalInput")
    selb_d = nc.dram_tensor("selb", [2, 128, 32], BF16, kind="ExternalInput")
    vmask_d = nc.dram_tensor("vmask", [128, 2 * D], BF16, kind="ExternalInput")
    ones_d = nc.dram_tensor("ones_sel", [128, N], BF16, kind="ExternalInput")
    outp_d = nc.dram_tensor("outp", [2, 128, L1], F32, kind="ExternalOutput")
    souts_d = nc.dram_tensor("souts", [N, 2, L1], F32, kind="ExternalOutput")
    vdram = nc.dram_tensor("vdram", [JSH, D], BF16)

    with tile.TileContext(nc) as tc:
        with (
            tc.tile_pool(name="const", bufs=1) as cpool,
            tc.tile_pool(name="vselp", bufs=1) as vpool,
            tc.tile_pool(name="work", bufs=4) as wpool,
            tc.tile_pool(name="tmps", bufs=8) as tpool,
            tc.tile_pool(name="ps_s", bufs=2, space="PSUM") as ps_pool,
            tc.tile_pool(name="ps_acc", bufs=1, space="PSUM") as pa_pool,
        ):
            # ---- load constants / inputs, spread over all DMA queues and
            # merged per tensor (fewer queue ops) so the K/Q projection
            # inputs land as early as possible ----
            xtT_a = cpool.tile([128, 2 * L1], BF16, name="xtT")
            xssT_a = cpool.tile([128, 2 * JSH], BF16, name="xssT")
            wqT_a = cpool.tile([128, 2 * D], BF16, name="wqT")
            wkT_a = cpool.tile([128, 2 * D], BF16, name="wkT")
            wvT_a = cpool.tile([128, 2 * D], BF16, name="wvT")
            selb = [cpool.tile([128, 32], BF16, name=f"selb{h}") for h in range(2)]
            vmask = cpool.tile([128, 2 * D], BF16, name="vmask")
            ones_sel = cpool.tile([128, N], BF16, name="ones_sel")
            bml = cpool.tile([128, 1], F32, name="bml")
            nc.vector.memset(bml[:], float(b_val))

            # ---- accumulators (also the warm-up target: quad 0's V-matmul
            # uses start=True, which clears whatever the warm-up wrote) ----
            ops = [
                pa_pool.tile([128, L1], F32, name=f"ops{eh}") for eh in range(2)
            ]
            sps = pa_pool.tile([16, 2, L1], F32, name="sps")

            # ---- PE warm-up burst: self-contained (memset inputs), runs at
            # t~0 so HAM flips to 8/8 and stays there until real matmuls flow
            wz = cpool.tile([128, L1], BF16, name="wz")
            nc.vector.memset(wz[:], 0.25)
            for k in range(12):
                nc.tensor.matmul(
                    ops[0][0:32, 0:256],
                    wz[:, 0:32],
                    wz[:, 0:256],
                    start=(k == 0),
                    stop=(k == 11),
                    skip_group_check=True,
                )
            # K-path inputs first (scalar+sync queues), Q-path in parallel
            # (vector+gpsimd), V and the small constants after.
            nc.scalar.dma_start(wkT_a[:], wkT_d[:])
            nc.sync.dma_start(xssT_a[:], xssT_d[:])
            nc.gpsimd.dma_start(wqT_a[:], wqT_d[:])
            nc.sync.dma_start(xtT_a[:], xtT_d[:])
            nc.sync.dma_start(wvT_a[:], wvT_d[:])
            for h in range(2):
                nc.gpsimd.dma_start(selb[h][:], selb_d[h])
            nc.sync.dma_start(vmask[:], vmask_d[:])
            nc.gpsimd.dma_start(ones_sel[:], ones_d[:])

            # ---- projections: K and Q first (the elementwise pipeline needs
            # them), V after ----
            QT = [cpool.tile([128, L1], BF16, name=f"QT{h}") for h in range(2)]
            KTn = [cpool.tile([128, JSH], F32, name=f"KTn{h}") for h in range(2)]
            for eh in range(2):
                psk = ps_pool.tile([128, JSH], F32, name="psk", tag="ps_s")
                for dh in range(2):
                    nc.tensor.matmul(
                        psk[:],
                        wkT_a[:, dh * D + eh * 128 : dh * D + (eh + 1) * 128],
                        xssT_a[:, dh * JSH : (dh + 1) * JSH],
                        start=(dh == 0),
                        stop=(dh == 1),
                    )
                nc.vector.tensor_scalar(KTn[eh][:], psk[:], -1.0, None, AL.mult)
                psq = ps_pool.tile([128, L1], F32, name="psq", tag="ps_s")
                for dh in range(2):
                    nc.tensor.matmul(
                        psq[:],
                        wqT_a[:, dh * D + eh * 128 : dh * D + (eh + 1) * 128],
                        xtT_a[:, dh * L1 : (dh + 1) * L1],
                        start=(dh == 0),
                        stop=(dh == 1),
                    )
                nc.vector.tensor_copy(QT[eh][:], psq[:])

            # ---- V projection -> DRAM -> per-pair broadcast+mask ----
            Vt = cpool.tile([128, D], BF16, name="Vt")
            psv = ps_pool.tile([128, D], F32, name="psv", tag="ps_s")
            for dh in range(2):
                nc.tensor.matmul(
                    psv[:],
                    xssT_a[:, dh * JSH : (dh + 1) * JSH],
                    wvT_a[:, dh * D : (dh + 1) * D],
                    start=(dh == 0),
                    stop=(dh == 1),
                )
            nc.vector.tensor_copy(Vt[:], psv[:])
            nc.sync.dma_start(vdram[:], Vt[:])

            V_sel = [
                vpool.tile([128, 2 * D], BF16, name=f"vs{k}") for k in range(NPAIR)
            ]

            def build_pair(k):
                vs2 = V_sel[k]
                for half in range(2):
                    gq = 2 * k + half
                    bsrc = (
                        vdram.ap()[4 * gq : 4 * gq + 4, :]
                        .unsqueeze(1)
                        .broadcast_to((4, 32, D))
                    )
                    eng = nc.sync if half == 0 else nc.gpsimd
                    eng.dma_start(vs2[:, half * D : (half + 1) * D], bsrc)
                nc.vector.tensor_tensor(vs2[:], vs2[:], vmask[:], op=AL.mult)

            for k in range(4):
                build_pair(k)

            # ---- main loop over 16 quad-pairs, software-pipelined ----
            state = {}

            def issue_v_units(q, half, jjs):
                """VectorE-path j's: bf16 t + two 32-col matmuls per j into
                the j's own 32-row PSUM slot (independent start/stop groups,
                so no engine ever gates another's slots)."""
                g = 2 * q + half
                psp = state[q]["ps"]
                s_jjs = _s_jjs(g)
                for jj in jjs:
                    if jj in s_jjs:
                        continue
                    j = 4 * g + jj
                    t = tpool.tile([128, 2, 512], BF16, name="tb", tag="tb")
                    for h in range(2):
                        nc.vector.tensor_scalar(
                            t[:, h, :],
                            QT[h][:],
                            KTn[h][:, j : j + 1],
                            0.0,
                            AL.add,
                            AL.max,
                        )
                        nc.tensor.matmul(
                            psp[32 * jj : 32 * jj + 32, half, :],
                            selb[h][:],
                            t[:, h, :],
                            start=(h == 0),
                            stop=(h == 1),
                            tile_position=(0, 32 * jj),
                            skip_group_check=True,
                        )

            def issue_s_units(q, half):
                """ScalarE-path j's: bf16 t via ACT, same 32-col matmuls,
                issued LAST in the quad so the PE never waits on ScalarE."""
                g = 2 * q + half
                psp = state[q]["ps"]
                s_jjs = _s_jjs(g)
                for jj in s_jjs:
                    j = 4 * g + jj
                    t = tpool.tile([128, 2, 512], BF16, name="ts", tag="ts")
                    for h in range(2):
                        nc.scalar.activation(
                            t[:, h, :],
                            QT[h][:],
                            AF.Relu,
                            bias=KTn[h][:, j : j + 1],
                            scale=1.0,
                        )
                        nc.tensor.matmul(
                            psp[32 * jj : 32 * jj + 32, half, :],
                            selb[h][:],
                            t[:, h, :],
                            start=(h == 0),
                            stop=(h == 1),
                            tile_position=(0, 32 * jj),
                            skip_group_check=True,
                        )

            def issue_exp(q, half=None):
                # p = exp(z/16 + b) over the pair's 2-bank PSUM span (or one
                # 512 half for the final drain)
                if "p" not in state[q]:
                    state[q]["p"] = wpool.tile(
                        [128, 2, L1], BF16, name="p", tag="p", bufs=3
                    )
                p = state[q]["p"]
                src = state[q]["ps"]
                if half is None:
                    nc.scalar.activation(
                        p[:], src[:], AF.Exp, bias=bml[:], scale=EXP_SCALE
                    )
                else:
                    nc.scalar.activation(
                        p[:, half, :],
                        src[:, half, :],
                        AF.Exp,
                        bias=bml[:],
                        scale=EXP_SCALE,
                    )

            def issue_pc(q, half=None):
                # pc = max(p, 1) = exp(relu(z/16 + b))
                if "pc" not in state[q]:
                    state[q]["pc"] = wpool.tile(
                        [128, 2, L1], BF16, name="pc", tag="pc", bufs=4
                    )
                pc = state[q]["pc"]
                p = state[q]["p"]
                if half is None:
                    nc.vector.tensor_scalar(pc[:], p[:], 1.0, None, AL.max)
                else:
                    nc.vector.tensor_scalar(
                        pc[:, half, :], p[:, half, :], 1.0, None, AL.max
                    )

            def issue_vmm_half(q, half, eh_outer=False):
                pc = state[q]["pc"]
                g = 2 * q + half
                ehs = [0, 1]
                for eh in ehs:
                    off = half * D + eh * 128
                    nc.tensor.matmul(
                        ops[eh][:],
                        V_sel[q][:, off : off + 128],
                        pc[:, half, :],
                        start=(g == 0),
                        stop=(g == NQUAD - 1),
                        skip_group_check=True,
                    )
                # denominator per quad half (PE writes can't cross PSUM banks)
                nc.tensor.matmul(
                    sps[:, half, :],
                    ones_sel[:, 0:N],
                    pc[:, half, :],
                    start=(q == 0),
                    stop=(q == NPAIR - 1),
                    skip_group_check=True,
                )

            def issue_vmm(q):
                issue_vmm_half(q, 0)
                issue_vmm_half(q, 1)
                del state[q]

            for q in range(NPAIR):
                if q + 4 < NPAIR:
                    build_pair(q + 4)
                state[q] = {
                    "ps": ps_pool.tile([128, 2, L1], F32, name="ps", tag="ps_s")
                }
                last = q == NPAIR - 1
                if q >= 1:
                    issue_exp(q - 1)
                issue_v_units(q, 0, (0, 1, 2))
                if q >= 1:
                    issue_pc(q - 1)
                issue_v_units(q, 0, (3,))
                issue_s_units(q, 0)
                if q >= 1:
                    # spread the previous pair's V-matmuls across the
                    # iteration so PE occupancy stays smooth (HAM never sees
                    # a >3.4us idle window mid-loop)
                    issue_vmm_half(q - 1, 0)
                if last:
                    issue_exp(q, 0)
                issue_v_units(q, 1, (0, 1, 2))
                if q >= 1:
                    issue_vmm_half(q - 1, 1)
                    del state[q - 1]
                if last:
                    issue_pc(q, 0)
                issue_v_units(q, 1, (3,))
                issue_s_units(q, 1)
                if last:
                    # drain the final pair per half so the output evacuation
                    # overlaps the second half's exp/pc chain
                    issue_vmm_half(q, 0)
                    issue_exp(q, 1)
                    issue_pc(q, 1)
                    issue_vmm_half(q, 1)

            # ---- evacuate + store ----
            for eh in range(2):
                ou = wpool.tile([128, L1], F32, name="ou", tag="ou", bufs=2)
                if eh == 0:
                    nc.vector.tensor_copy(ou[:], ops[eh][:])
                else:
                    nc.scalar.copy(ou[:], ops[eh][:])
                nc.sync.dma_start(outp_d[eh], ou[:])
            so = wpool.tile([16, 2, L1], F32, name="so")
            nc.scalar.copy(so[:], sps[:])
            nc.sync.dma_start(souts_d[:], so[:])

    nc.compile()
    return nc


_CACHE: dict = {}


def _get_graph(b_val: float):
    key = round(float(b_val), 10)
    if key not in _CACHE:
        _CACHE[key] = _build(float(b_val))
    return _CACHE[key]


def _host_prep(x_source, x_target, Wq, Wk, Wv, w_mlp):
    """Build per-core input maps (numpy, bf16/fp8)."""
    w_full = np.tile(np.asarray(w_mlp, np.float32), D // G)  # w_full[d] = w[d%16]
    # bf16 sel: [2 h][128 part, 32 cols], col = group(d); with the x8 in
    # Wq/Wk this emits z*16 (EXP_SCALE undoes it)
    selb = np.zeros((2, 128, 32), np.float32)
    for h in range(2):
        for dl in range(128):
            d = 128 * h + dl
            selb[h, dl, d // G] = SEL_SCALE * w_full[d]
    # V_sel mask: row p = 32*jj + s (s<16 valid), col e: keep if e%16 == s
    vmask = np.zeros((128, 2 * D), np.float32)
    for p in range(128):
        s = p % 32
        if s < 16:
            vmask[p, s::G] = 1.0
    # S selector: row p = 32*jj + s -> column s (s < 16)
    ones_sel = np.zeros((128, N), np.float32)
    for p in range(128):
        s = p % 32
        if s < 16:
            ones_sel[p, s] = 1.0

    def split_h(a):  # (256, X) -> (128, 2X): [dl, h*X+x] = a[128h+dl, x]
        X = a.shape[1]
        return np.ascontiguousarray(
            a.reshape(2, 128, X).transpose(1, 0, 2)
        ).reshape(128, 2 * X)

    wq_b = split_h(SCALE_T * np.asarray(Wq, np.float32).T).astype(BF)
    wk_b = split_h(SCALE_T * np.asarray(Wk, np.float32).T).astype(BF)
    wv_b = split_h(np.asarray(Wv, np.float32).T).astype(BF)
    selb_b = selb.astype(BF)
    vmask_b = vmask.astype(BF)
    ones_b = ones_sel.astype(BF)

    xtT = [
        split_h(np.asarray(x_target[b], np.float32).T).astype(BF)
        for b in range(B)
    ]
    xsT = [np.asarray(x_source[b], np.float32).T for b in range(B)]
    in_maps = []
    for core in range(NCORES):
        b, jq = divmod(core, 4)
        j0 = jq * JSH
        xssT = split_h(xsT[b][:, j0 : j0 + JSH]).astype(BF)
        in_maps.append(
            {
                "xtT": xtT[b],
                "xssT": xssT,
                "wqT": wq_b,
                "wkT": wk_b,
                "wvT": wv_b,
                "selb": selb_b,
                "vmask": vmask_b,
                "ones_sel": ones_b,
            }
        )
    return in_maps


def _host_gather(results):
    """Sum partials over j-shards, normalize, reshape to (B, L1, D)."""
    out = np.empty((B, L1, D), np.float32)
    for b in range(B):
        cores = [b * 4 + jq for jq in range(4)]
        U = sum(
            results[c]["outp"].reshape(D, L1).astype(np.float64) for c in cores
        )  # (e, i)
        S = sum(
            results[c]["souts"].sum(axis=1).astype(np.float64) for c in cores
        )  # (nn, i): paired denominator halves summed
        att = U / S[np.arange(D) % N, :]  # (e, i)
        out[b] = att.T.astype(np.float32)
    return out


def run(inputs, trace=False, **kwargs):
    nc = _get_graph(float(np.asarray(inputs["b_mlp"]).reshape(-1)[0]))
    in_maps = _host_prep(
        inputs["x_source"],
        inputs["x_target"],
        inputs["Wq"],
        inputs["Wk"],
        inputs["Wv"],
        inputs["w_mlp"],
    )
    res = run_bass_kernel_spmd(
        nc, in_maps, core_ids=list(range(NCORES)), trace=trace, **kwargs
    )
    return _host_gather(res.results), res


def kernel(**inputs) -> np.ndarray:
    out, _ = run(inputs, trace=False)
    return out


# revision 56
# speedup vs baseline: 1.0155x; 1.0155x over previous
"""Trainium2 Bass kernel for grouped vector attention (sparse_attention).

Reference computation (B=2, L1=L2=512, D=256, g=16, n=16):
    Q = x_target @ Wq.T ; K = x_source @ Wk.T ; V = x_source @ Wv.T
    diff = Q.reshape(B,L1,1,n,g) - K.reshape(B,1,L2,n,g)
    scores = relu(einsum('bijng,g->bijn', relu(diff), w_mlp) + b_mlp)
    att = softmax(scores, axis=2)                      # over L2
    out = einsum('bijn,bjgn->bign', att, V.reshape(B,L2,g,n)).reshape(B,L1,D)

Sharding: 8 cores = 2 batches x 4 L2(j)-quarters. Each core handles all 512
queries against its 128 source positions and produces partial (unnormalized)
outputs + partial softmax denominators; the host sums the 4 partials per
batch and divides. Sharding over j (not i) means the exp'd scores come out
with j on partitions — exactly what the att@V contraction needs, so there is
no on-chip transpose anywhere.

Per-core pipeline, per source position j (128 of them):
  - t[d,(h,i)] = relu(8Q[i,d] - 8K[j,d]) with d on partitions (Wq/Wk are
    pre-scaled x8 on host). Each j is assigned to ONE elementwise engine:
      ScalarE j's  -> t in fp8e4m3 (ACT pays no fp8 penalty), scores via ONE
                      fp8 DoubleRow matmul (contracts all 256 d at once,
                      128-wide sel variant per jj slot since walrus rejects
                      DoubleRow + tile_position col offsets)
      VectorE j's  -> t in bf16 (DVE 2x mode; fp8-out measured +105ns), via
                      two bf16 matmuls with 32-col sel at tile_position.
    Scores land x16 scaled in the quad's 32-row PSUM slot.
  - per PAIR of quads: one exp over the [128,1024] 2-bank PSUM span
    (p = exp(z/16 + b)), one pc = max(p,1) on DVE, one paired denominator
    matmul (ones_sel.T @ pc_pair -> [16,1024], host sums the halves).
  - V path bf16: V_sel[g][32*jj+nn, e] = V[4g+jj, e]*(e%16==nn) built per
    quad-pair by broadcast-DMA from a DRAM copy of V + masked mult;
    out_partial[e,i] += V_sel.T @ pc  (PSUM accumulation over all quads).
  - exp/pc/V-matmul issue is software-pipelined one pair behind the
    elementwise+score stream so no engine queue head-of-line blocks.
"""

import numpy as np

import concourse.bass as bass
import concourse.bacc as bacc
import concourse.tile as tile
import concourse.mybir as mybir
from concourse.bass_utils import run_bass_kernel_spmd

import ml_dtypes

F32 = mybir.dt.float32
BF16 = mybir.dt.bfloat16
FP8 = mybir.dt.float8e4
AL = mybir.AluOpType
AF = mybir.ActivationFunctionType
DR = mybir.MatmulPerfMode.DoubleRow

B, L1, L2, D = 2, 512, 512, 256
G = 16           # group size (d_group)
N = 16           # number of groups
NCORES = 8
JSH = 128        # source positions per core (L2 / 4)
NQUAD = 32       # 32 quads of 4 source positions
NPAIR = 16       # pairs of quads
BF = ml_dtypes.bfloat16
F8 = ml_dtypes.float8_e4m3

SCALE_T = 8.0    # folded into Wq/Wk on host: t = relu(8q-8k) in e4m3 range
SEL_SCALE = 2.0  # fp8 sel = 2*w so fp8 weights stay in normal range
EXP_SCALE = 1.0 / (SCALE_T * SEL_SCALE)


def _s_jjs(g):
    """Which jj slots of quad g go to the ScalarE elementwise path.
    2.5 S-j's per pair (5 ACT units) balances ScalarE (707ns/unit + the
    paired exp) against VectorE (345ns/unit + pc + V_sel masking). S-path
    work is issued LAST in each quad so the PE never waits on ScalarE."""
    return (2, 3) if g % 8 == 3 else (3,)


def _build(b_val: float):
    """Build + compile the per-core Bass graph. Same graph for all 8 cores."""
    nc = bacc.Bacc(
        "TRN2", target_bir_lowering=False, debug=False, enable_asserts=False
    )

    # ---- DRAM parameters (per-core shards, host-prepped) ----
    xtT_d = nc.dram_tensor("xtT", [128, 2 * L1], BF16, kind="ExternalInput")
    xssT_d = nc.dram_tensor("xssT", [128, 2 * JSH], BF16, kind="ExternalInput")
    wqT_d = nc.dram_tensor("wqT", [128, 2 * D], BF16, kind="ExternalInput")
    wkT_d = nc.dram_tensor("wkT", [128, 2 * D], BF16, kind="ExternalInput")
    wvT_d = nc.dram_tensor("wvT", [128, 2 * D], BF16, kind="ExternalInput")
    selb_d = nc.dram_tensor("selb", [2, 128, 32], BF16, kind="ExternalInput")
    vmask_d = nc.dram_tensor("vmask", [128, 2 * D], BF16, kind="ExternalInput")
    ones_d = nc.dram_tensor("ones_sel", [128, N], BF16, kind="ExternalInput")
    outp_d = nc.dram_tensor("outp", [2, 128, L1], F32, kind="ExternalOutput")
    souts_d = nc.dram_tensor("souts", [N, 2, L1], F32, kind="ExternalOutput")
    vdram = nc.dram_tensor("vdram", [JSH, D], BF16)

    with tile.TileContext(nc) as tc:
        with (
            tc.tile_pool(name="const", bufs=1) as cpool,
            tc.tile_pool(name="vselp", bufs=1) as vpool,
            tc.tile_pool(name="work", bufs=4) as wpool,
            tc.tile_pool(name="tmps", bufs=8) as tpool,
            tc.tile_pool(name="ps_s", bufs=2, space="PSUM") as ps_pool,
            tc.tile_pool(name="ps_acc", bufs=1, space="PSUM") as pa_pool,
        ):
            # ---- load constants / inputs, spread over all DMA queues and
            # merged per tensor (fewer queue ops) so the K/Q projection
            # inputs land as early as possible ----
            xtT_a = cpool.tile([128, 2 * L1], BF16, name="xtT")
            xssT_a = cpool.tile([128, 2 * JSH], BF16, name="xssT")
            wqT_a = cpool.tile([128, 2 * D], BF16, name="wqT")
            wkT_a = cpool.tile([128, 2 * D], BF16, name="wkT")
            wvT_a = cpool.tile([128, 2 * D], BF16, name="wvT")
            selb = [cpool.tile([128, 32], BF16, name=f"selb{h}") for h in range(2)]
            vmask = cpool.tile([128, 2 * D], BF16, name="vmask")
            ones_sel = cpool.tile([128, N], BF16, name="ones_sel")
            bml = cpool.tile([128, 1], F32, name="bml")
            nc.vector.memset(bml[:], float(b_val))

            # ---- accumulators (also the warm-up target: quad 0's V-matmul
            # uses start=True, which clears whatever the warm-up wrote) ----
            ops = [
                pa_pool.tile([128, L1], F32, name=f"ops{eh}") for eh in range(2)
            ]
            sps = pa_pool.tile([16, 2, L1], F32, name="sps")

            # ---- PE warm-up burst: self-contained (memset inputs), runs at
            # t~0 so HAM flips to 8/8 and stays there until real matmuls flow
            wz = cpool.tile([128, L1], BF16, name="wz")
            nc.vector.memset(wz[:], 0.25)
            for k in range(12):
                nc.tensor.matmul(
                    ops[0][0:32, 0:256],
                    wz[:, 0:32],
                    wz[:, 0:256],
                    start=(k == 0),
                    stop=(k == 11),
                    skip_group_check=True,
                )
            # K-path inputs first (scalar+sync queues), Q-path in parallel
            # (vector+gpsimd), V and the small constants after.
            nc.scalar.dma_start(wkT_a[:], wkT_d[:])
            nc.sync.dma_start(xssT_a[:], xssT_d[:])
            nc.gpsimd.dma_start(wqT_a[:], wqT_d[:])
            nc.sync.dma_start(xtT_a[:], xtT_d[:])
            nc.sync.dma_start(wvT_a[:], wvT_d[:])
            for h in range(2):
                nc.gpsimd.dma_start(selb[h][:], selb_d[h])
            nc.sync.dma_start(vmask[:], vmask_d[:])
            nc.gpsimd.dma_start(ones_sel[:], ones_d[:])

            # ---- projections: K and Q first (the elementwise pipeline needs
            # them), V after ----
            QT = [cpool.tile([128, L1], BF16, name=f"QT{h}") for h in range(2)]
            KTn = [cpool.tile([128, JSH], F32, name=f"KTn{h}") for h in range(2)]
            for eh in range(2):
                psk = ps_pool.tile([128, JSH], F32, name="psk", tag="ps_s")
                for dh in range(2):
                    nc.tensor.matmul(
                        psk[:],
                        wkT_a[:, dh * D + eh * 128 : dh * D + (eh + 1) * 128],
                        xssT_a[:, dh * JSH : (dh + 1) * JSH],
                        start=(dh == 0),
                        stop=(dh == 1),
                    )
                nc.vector.tensor_scalar(KTn[eh][:], psk[:], -1.0, None, AL.mult)
                psq = ps_pool.tile([128, L1], F32, name="psq", tag="ps_s")
                for dh in range(2):
                    nc.tensor.matmul(
                        psq[:],
                        wqT_a[:, dh * D + eh * 128 : dh * D + (eh + 1) * 128],
                        xtT_a[:, dh * L1 : (dh + 1) * L1],
                        start=(dh == 0),
                        stop=(dh == 1),
                    )
                nc.vector.tensor_copy(QT[eh][:], psq[:])

            # ---- V projection -> DRAM -> per-pair broadcast+mask ----
            Vt = cpool.tile([128, D], BF16, name="Vt")
            psv = ps_pool.tile([128, D], F32, name="psv", tag="ps_s")
            for dh in range(2):
                nc.tensor.matmul(
                    psv[:],
                    xssT_a[:, dh * JSH : (dh + 1) * JSH],
                    wvT_a[:, dh * D : (dh + 1) * D],
                    start=(dh == 0),
                    stop=(dh == 1),
                )
            nc.scalar.copy(Vt[:], psv[:])
            nc.sync.dma_start(vdram[:], Vt[:])

            V_sel = [
                vpool.tile([128, 2 * D], BF16, name=f"vs{k}") for k in range(NPAIR)
            ]

            def build_pair(k):
                vs2 = V_sel[k]
                for half in range(2):
                    gq = 2 * k + half
                    bsrc = (
                        vdram.ap()[4 * gq : 4 * gq + 4, :]
                        .unsqueeze(1)
                        .broadcast_to((4, 32, D))
                    )
                    eng = nc.sync if half == 0 else nc.gpsimd
                    eng.dma_start(vs2[:, half * D : (half + 1) * D], bsrc)
                nc.vector.tensor_tensor(vs2[:], vs2[:], vmask[:], op=AL.mult)

            for k in range(2):
                build_pair(k)

            # ---- main loop over 16 quad-pairs, software-pipelined ----
            state = {}

            def issue_v_units(q, half, jjs):
                """VectorE-path j's: bf16 t + two 32-col matmuls per j into
                the j's own 32-row PSUM slot (independent start/stop groups,
                so no engine ever gates another's slots)."""
                g = 2 * q + half
                psp = state[q]["ps"]
                s_jjs = _s_jjs(g)
                for jj in jjs:
                    if jj in s_jjs:
                        continue
                    j = 4 * g + jj
                    t = tpool.tile([128, 2, 512], BF16, name="tb", tag="tb")
                    for h in range(2):
                        nc.vector.tensor_scalar(
                            t[:, h, :],
                            QT[h][:],
                            KTn[h][:, j : j + 1],
                            0.0,
                            AL.add,
                            AL.max,
                        )
                        nc.tensor.matmul(
                            psp[32 * jj : 32 * jj + 32, half, :],
                            selb[h][:],
                            t[:, h, :],
                            start=(h == 0),
                            stop=(h == 1),
                            tile_position=(0, 32 * jj),
                            skip_group_check=True,
                        )

            def issue_s_units(q, half):
                """ScalarE-path j's: bf16 t via ACT, same 32-col matmuls,
                issued LAST in the quad so the PE never waits on ScalarE."""
                g = 2 * q + half
                psp = state[q]["ps"]
                s_jjs = _s_jjs(g)
                for jj in s_jjs:
                    j = 4 * g + jj
                    t = tpool.tile([128, 2, 512], BF16, name="ts", tag="ts")
                    for h in range(2):
                        nc.scalar.activation(
                            t[:, h, :],
                            QT[h][:],
                            AF.Relu,
                            bias=KTn[h][:, j : j + 1],
                            scale=1.0,
                        )
                        nc.tensor.matmul(
                            psp[32 * jj : 32 * jj + 32, half, :],
                            selb[h][:],
                            t[:, h, :],
                            start=(h == 0),
                            stop=(h == 1),
                            tile_position=(0, 32 * jj),
                            skip_group_check=True,
                        )

            def issue_exp(q, half=None):
                # p = exp(z/16 + b) over the pair's 2-bank PSUM span (or one
                # 512 half for the final drain)
                if "p" not in state[q]:
                    state[q]["p"] = wpool.tile(
                        [128, 2, L1], BF16, name="p", tag="p", bufs=3
                    )
                p = state[q]["p"]
                src = state[q]["ps"]
                if half is None:
                    nc.scalar.activation(
                        p[:], src[:], AF.Exp, bias=bml[:], scale=EXP_SCALE
                    )
                else:
                    nc.scalar.activation(
                        p[:, half, :],
                        src[:, half, :],
                        AF.Exp,
                        bias=bml[:],
                        scale=EXP_SCALE,
                    )

            def issue_pc(q, half=None):
                # pc = max(p, 1) = exp(relu(z/16 + b))
                if "pc" not in state[q]:
                    state[q]["pc"] = wpool.tile(
                        [128, 2, L1], BF16, name="pc", tag="pc", bufs=4
                    )
                pc = state[q]["pc"]
                p = state[q]["p"]
                if half is None:
                    nc.vector.tensor_scalar(pc[:], p[:], 1.0, None, AL.max)
                else:
                    nc.vector.tensor_scalar(
                        pc[:, half, :], p[:, half, :], 1.0, None, AL.max
                    )

            def issue_vmm_half(q, half, eh_outer=False):
                pc = state[q]["pc"]
                g = 2 * q + half
                ehs = [0, 1]
                for eh in ehs:
                    off = half * D + eh * 128
                    nc.tensor.matmul(
                        ops[eh][:],
                        V_sel[q][:, off : off + 128],
                        pc[:, half, :],
                        start=(g == 0),
                        stop=(g == NQUAD - 1),
                        skip_group_check=True,
                    )
                # denominator per quad half (PE writes can't cross PSUM banks)
                nc.tensor.matmul(
                    sps[:, half, :],
                    ones_sel[:, 0:N],
                    pc[:, half, :],
                    start=(q == 0),
                    stop=(q == NPAIR - 1),
                    skip_group_check=True,
                )

            def issue_vmm(q):
                issue_vmm_half(q, 0)
                issue_vmm_half(q, 1)
                del state[q]

            for q in range(NPAIR):
                if q + 2 < NPAIR:
                    build_pair(q + 2)
                state[q] = {
                    "ps": ps_pool.tile([128, 2, L1], F32, name="ps", tag="ps_s")
                }
                last = q == NPAIR - 1
                if q >= 1:
                    issue_exp(q - 1)
                issue_v_units(q, 0, (0, 1, 2))
                if q >= 1:
                    issue_pc(q - 1)
                issue_v_units(q, 0, (3,))
                issue_s_units(q, 0)
                if q >= 1:
                    # spread the previous pair's V-matmuls across the
                    # iteration so PE occupancy stays smooth (HAM never sees
                    # a >3.4us idle window mid-loop)
                    issue_vmm_half(q - 1, 0)
                if last:
                    issue_exp(q, 0)
                issue_v_units(q, 1, (0, 1, 2))
                if q >= 1:
                    issue_vmm_half(q - 1, 1)
                    del state[q - 1]
                if last:
                    issue_pc(q, 0)
                issue_v_units(q, 1, (3,))
                issue_s_units(q, 1)
                if last:
                    # drain the final pair per half so the output evacuation
                    # overlaps the second half's exp/pc chain
                    issue_vmm_half(q, 0)
                    issue_exp(q, 1)
                    issue_pc(q, 1)
                    issue_vmm_half(q, 1)

            # ---- evacuate + store ----
            for eh in range(2):
                ou = wpool.tile([128, L1], F32, name="ou", tag="ou", bufs=2)
                if eh == 0:
                    nc.vector.tensor_copy(ou[:], ops[eh][:])
                else:
                    nc.scalar.copy(ou[:], ops[eh][:])
                nc.sync.dma_start(outp_d[eh], ou[:])
            so = wpool.tile([16, 2, L1], F32, name="so")
            nc.scalar.copy(so[:], sps[:])
            nc.sync.dma_start(souts_d[:], so[:])

    nc.compile()
    return nc


_CACHE: dict = {}


def _get_graph(b_val: float):
    key = round(float(b_val), 10)
    if key not in _CACHE:
        _CACHE[key] = _build(float(b_val))
    return _CACHE[key]


def _host_prep(x_source, x_target, Wq, Wk, Wv, w_mlp):
    """Build per-core input maps (numpy, bf16/fp8)."""
    w_full = np.tile(np.asarray(w_mlp, np.float32), D // G)  # w_full[d] = w[d%16]
    # bf16 sel: [2 h][128 part, 32 cols], col = group(d); with the x8 in
    # Wq/Wk this emits z*16 (EXP_SCALE undoes it)
    selb = np.zeros((2, 128, 32), np.float32)
    for h in range(2):
        for dl in range(128):
            d = 128 * h + dl
            selb[h, dl, d // G] = SEL_SCALE * w_full[d]
    # V_sel mask: row p = 32*jj + s (s<16 valid), col e: keep if e%16 == s
    vmask = np.zeros((128, 2 * D), np.float32)
    for p in range(128):
        s = p % 32
        if s < 16:
            vmask[p, s::G] = 1.0
    # S selector: row p = 32*jj + s -> column s (s < 16)
    ones_sel = np.zeros((128, N), np.float32)
    for p in range(128):
        s = p % 32
        if s < 16:
            ones_sel[p, s] = 1.0

    def split_h(a):  # (256, X) -> (128, 2X): [dl, h*X+x] = a[128h+dl, x]
        X = a.shape[1]
        return np.ascontiguousarray(
            a.reshape(2, 128, X).transpose(1, 0, 2)
        ).reshape(128, 2 * X)

    wq_b = split_h(SCALE_T * np.asarray(Wq, np.float32).T).astype(BF)
    wk_b = split_h(SCALE_T * np.asarray(Wk, np.float32).T).astype(BF)
    wv_b = split_h(np.asarray(Wv, np.float32).T).astype(BF)
    selb_b = selb.astype(BF)
    vmask_b = vmask.astype(BF)
    ones_b = ones_sel.astype(BF)

    xtT = [
        split_h(np.asarray(x_target[b], np.float32).T).astype(BF)
        for b in range(B)
    ]
    xsT = [np.asarray(x_source[b], np.float32).T for b in range(B)]
    in_maps = []
    for core in range(NCORES):
        b, jq = divmod(core, 4)
        j0 = jq * JSH
        xssT = split_h(xsT[b][:, j0 : j0 + JSH]).astype(BF)
        in_maps.append(
            {
                "xtT": xtT[b],
                "xssT": xssT,
                "wqT": wq_b,
                "wkT": wk_b,
                "wvT": wv_b,
                "selb": selb_b,
                "vmask": vmask_b,
                "ones_sel": ones_b,
            }
        )
    return in_maps


def _host_gather(results):
    """Sum partials over j-shards, normalize, reshape to (B, L1, D)."""
    out = np.empty((B, L1, D), np.float32)
    for b in range(B):
        cores = [b * 4 + jq for jq in range(4)]
        U = sum(
            results[c]["outp"].reshape(D, L1).astype(np.float64) for c in cores
        )  # (e, i)
        S = sum(
            results[c]["souts"].sum(axis=1).astype(np.float64) for c in cores
        )  # (nn, i): paired denominator halves summed
        att = U / S[np.arange(D) % N, :]  # (e, i)
        out[b] = att.T.astype(np.float32)
    return out


def run(inputs, trace=False, **kwargs):
    nc = _get_graph(float(np.asarray(inputs["b_mlp"]).reshape(-1)[0]))
    in_maps = _host_prep(
        inputs["x_source"],
        inputs["x_target"],
        inputs["Wq"],
        inputs["Wk"],
        inputs["Wv"],
        inputs["w_mlp"],
    )
    res = run_bass_kernel_spmd(
        nc, in_maps, core_ids=list(range(NCORES)), trace=trace, **kwargs
    )
    return _host_gather(res.results), res


def kernel(**inputs) -> np.ndarray:
    out, _ = run(inputs, trace=False)
    return out


# revision 57
# speedup vs baseline: 1.0333x; 1.0175x over previous
"""Trainium2 Bass kernel for grouped vector attention (sparse_attention).

Reference computation (B=2, L1=L2=512, D=256, g=16, n=16):
    Q = x_target @ Wq.T ; K = x_source @ Wk.T ; V = x_source @ Wv.T
    diff = Q.reshape(B,L1,1,n,g) - K.reshape(B,1,L2,n,g)
    scores = relu(einsum('bijng,g->bijn', relu(diff), w_mlp) + b_mlp)
    att = softmax(scores, axis=2)                      # over L2
    out = einsum('bijn,bjgn->bign', att, V.reshape(B,L2,g,n)).reshape(B,L1,D)

Sharding: 8 cores = 2 batches x 4 L2(j)-quarters. Each core handles all 512
queries against its 128 source positions and produces partial (unnormalized)
outputs + partial softmax denominators; the host sums the 4 partials per
batch and divides. Sharding over j (not i) means the exp'd scores come out
with j on partitions — exactly what the att@V contraction needs, so there is
no on-chip transpose anywhere.

Per-core pipeline, per source position j (128 of them):
  - t[d,(h,i)] = relu(8Q[i,d] - 8K[j,d]) bf16 with d on partitions (Wq/Wk
    pre-scaled x8 on host). Each j is assigned to ONE elementwise engine
    (~4.5 of 16 half-units per pair on ScalarE, rest on VectorE; ScalarE
    work issued last in each quad so the PE never waits on it). Scores via
    two 32-col matmuls per j into the j's own 32-row PSUM slot
    (tile_position packing, independent start/stop groups), x16 scaled.
  - per PAIR of quads: one exp over the [128,1024] 2-bank PSUM span
    (p = exp(z/16 + b)), one pc = max(p,1) on DVE, one paired denominator
    matmul (ones_sel.T @ pc_pair -> [16,1024], host sums the halves).
  - V path bf16: V_sel[g][32*jj+nn, e] = V[4g+jj, e]*(e%16==nn) built per
    quad-pair by broadcast-DMA from a DRAM copy of V + masked mult;
    out_partial[e,i] += V_sel.T @ pc  (PSUM accumulation over all quads).
  - exp/pc/V-matmul issue is software-pipelined one pair behind the
    elementwise+score stream so no engine queue head-of-line blocks.
"""

import numpy as np

import concourse.bass as bass
import concourse.bacc as bacc
import concourse.tile as tile
import concourse.mybir as mybir
from concourse.bass_utils import run_bass_kernel_spmd

import ml_dtypes

F32 = mybir.dt.float32
BF16 = mybir.dt.bfloat16
FP8 = mybir.dt.float8e4
AL = mybir.AluOpType
AF = mybir.ActivationFunctionType
DR = mybir.MatmulPerfMode.DoubleRow

B, L1, L2, D = 2, 512, 512, 256
G = 16           # group size (d_group)
N = 16           # number of groups
NCORES = 8
JSH = 128        # source positions per core (L2 / 4)
NQUAD = 32       # 32 quads of 4 source positions
NPAIR = 16       # pairs of quads
BF = ml_dtypes.bfloat16
F8 = ml_dtypes.float8_e4m3

SCALE_T = 8.0    # folded into Wq/Wk on host: t = relu(8q-8k) in e4m3 range
SEL_SCALE = 2.0  # fp8 sel = 2*w so fp8 weights stay in normal range
EXP_SCALE = 1.0 / (SCALE_T * SEL_SCALE)


def _s_jjs(g):
    """Which jj slots of quad g go to the ScalarE elementwise path.
    2.5 S-j's per pair (5 ACT units) balances ScalarE (707ns/unit + the
    paired exp) against VectorE (345ns/unit + pc + V_sel masking). S-path
    work is issued LAST in each quad so the PE never waits on ScalarE."""
    return (2, 3) if g % 8 == 3 else (3,)


def _build(b_val: float):
    """Build + compile the per-core Bass graph. Same graph for all 8 cores."""
    nc = bacc.Bacc(
        "TRN2", target_bir_lowering=False, debug=False, enable_asserts=False
    )

    # ---- DRAM parameters (per-core shards, host-prepped) ----
    xtT_d = nc.dram_tensor("xtT", [128, 2 * L1], BF16, kind="ExternalInput")
    xssT_d = nc.dram_tensor("xssT", [128, 2 * JSH], BF16, kind="ExternalInput")
    wqT_d = nc.dram_tensor("wqT", [128, 2 * D], BF16, kind="ExternalInput")
    wkT_d = nc.dram_tensor("wkT", [128, 2 * D], BF16, kind="ExternalInput")
    wvT_d = nc.dram_tensor("wvT", [128, 2 * D], BF16, kind="ExternalInput")
    selb_d = nc.dram_tensor("selb", [2, 128, 32], BF16, kind="ExternalInput")
    vmask_d = nc.dram_tensor("vmask", [128, 2 * D], BF16, kind="ExternalInput")
    ones_d = nc.dram_tensor("ones_sel", [128, N], BF16, kind="ExternalInput")
    outp_d = nc.dram_tensor("outp", [2, 128, L1], F32, kind="ExternalOutput")
    souts_d = nc.dram_tensor("souts", [N, 2, L1], F32, kind="ExternalOutput")
    vdram = nc.dram_tensor("vdram", [JSH, D], BF16)

    with tile.TileContext(nc) as tc:
        with (
            tc.tile_pool(name="const", bufs=1) as cpool,
            tc.tile_pool(name="vselp", bufs=1) as vpool,
            tc.tile_pool(name="work", bufs=4) as wpool,
            tc.tile_pool(name="tmps", bufs=8) as tpool,
            tc.tile_pool(name="ps_s", bufs=2, space="PSUM") as ps_pool,
            tc.tile_pool(name="ps_acc", bufs=1, space="PSUM") as pa_pool,
        ):
            # ---- load constants / inputs, spread over all DMA queues and
            # merged per tensor (fewer queue ops) so the K/Q projection
            # inputs land as early as possible ----
            xtT_a = cpool.tile([128, 2 * L1], BF16, name="xtT")
            xssT_a = cpool.tile([128, 2 * JSH], BF16, name="xssT")
            wqT_a = cpool.tile([128, 2 * D], BF16, name="wqT")
            wkT_a = cpool.tile([128, 2 * D], BF16, name="wkT")
            wvT_a = cpool.tile([128, 2 * D], BF16, name="wvT")
            selb = [cpool.tile([128, 32], BF16, name=f"selb{h}") for h in range(2)]
            vmask = cpool.tile([128, 2 * D], BF16, name="vmask")
            ones_sel = cpool.tile([128, N], BF16, name="ones_sel")
            bml = cpool.tile([128, 1], F32, name="bml")
            nc.vector.memset(bml[:], float(b_val))

            # ---- accumulators (also the warm-up target: quad 0's V-matmul
            # uses start=True, which clears whatever the warm-up wrote) ----
            ops = [
                pa_pool.tile([128, L1], F32, name=f"ops{eh}") for eh in range(2)
            ]
            sps = pa_pool.tile([16, 2, L1], F32, name="sps")

            # ---- PE warm-up burst: self-contained (memset inputs), runs at
            # t~0 so HAM flips to 8/8 and stays there until real matmuls flow
            wz = cpool.tile([128, L1], BF16, name="wz")
            nc.vector.memset(wz[:], 0.25)
            for k in range(12):
                nc.tensor.matmul(
                    ops[0][0:32, 0:256],
                    wz[:, 0:32],
                    wz[:, 0:256],
                    start=(k == 0),
                    stop=(k == 11),
                    skip_group_check=True,
                )
            # K-path inputs first (scalar+sync queues), Q-path in parallel
            # (vector+gpsimd), V and the small constants after.
            nc.scalar.dma_start(wkT_a[:], wkT_d[:])
            nc.sync.dma_start(xssT_a[:], xssT_d[:])
            nc.gpsimd.dma_start(wqT_a[:], wqT_d[:])
            nc.sync.dma_start(xtT_a[:], xtT_d[:])
            nc.sync.dma_start(wvT_a[:], wvT_d[:])
            for h in range(2):
                nc.gpsimd.dma_start(selb[h][:], selb_d[h])
            nc.sync.dma_start(vmask[:], vmask_d[:])
            nc.gpsimd.dma_start(ones_sel[:], ones_d[:])

            # ---- projections: K and Q first (the elementwise pipeline needs
            # them), V after ----
            QT = [cpool.tile([128, L1], BF16, name=f"QT{h}") for h in range(2)]
            KTn = [cpool.tile([128, JSH], F32, name=f"KTn{h}") for h in range(2)]
            for eh in range(2):
                psk = ps_pool.tile([128, JSH], F32, name="psk", tag="ps_s")
                for dh in range(2):
                    nc.tensor.matmul(
                        psk[:],
                        wkT_a[:, dh * D + eh * 128 : dh * D + (eh + 1) * 128],
                        xssT_a[:, dh * JSH : (dh + 1) * JSH],
                        start=(dh == 0),
                        stop=(dh == 1),
                    )
                nc.vector.tensor_scalar(KTn[eh][:], psk[:], -1.0, None, AL.mult)
                psq = ps_pool.tile([128, L1], F32, name="psq", tag="ps_s")
                for dh in range(2):
                    nc.tensor.matmul(
                        psq[:],
                        wqT_a[:, dh * D + eh * 128 : dh * D + (eh + 1) * 128],
                        xtT_a[:, dh * L1 : (dh + 1) * L1],
                        start=(dh == 0),
                        stop=(dh == 1),
                    )
                nc.vector.tensor_copy(QT[eh][:], psq[:])

            # ---- V projection -> DRAM -> per-pair broadcast+mask ----
            Vt = cpool.tile([128, D], BF16, name="Vt")
            psv = ps_pool.tile([128, D], F32, name="psv", tag="ps_s")
            for dh in range(2):
                nc.tensor.matmul(
                    psv[:],
                    xssT_a[:, dh * JSH : (dh + 1) * JSH],
                    wvT_a[:, dh * D : (dh + 1) * D],
                    start=(dh == 0),
                    stop=(dh == 1),
                )
            nc.scalar.copy(Vt[:], psv[:])
            nc.sync.dma_start(vdram[:], Vt[:])

            V_sel = [
                vpool.tile([128, 2 * D], BF16, name=f"vs{k}") for k in range(NPAIR)
            ]

            def build_pair(k):
                vs2 = V_sel[k]
                for half in range(2):
                    gq = 2 * k + half
                    bsrc = (
                        vdram.ap()[4 * gq : 4 * gq + 4, :]
                        .unsqueeze(1)
                        .broadcast_to((4, 32, D))
                    )
                    eng = nc.sync if half == 0 else nc.gpsimd
                    eng.dma_start(vs2[:, half * D : (half + 1) * D], bsrc)
                nc.vector.tensor_tensor(vs2[:], vs2[:], vmask[:], op=AL.mult)

            for k in range(2):
                build_pair(k)

            # ---- main loop over 16 quad-pairs, software-pipelined ----
            state = {}

            def issue_v_units(q, half, jjs):
                """VectorE-path j's: bf16 t + two 32-col matmuls per j into
                the j's own 32-row PSUM slot (independent start/stop groups,
                so no engine ever gates another's slots)."""
                g = 2 * q + half
                psp = state[q]["ps"]
                s_jjs = _s_jjs(g)
                for jj in jjs:
                    if jj in s_jjs:
                        continue
                    j = 4 * g + jj
                    t = tpool.tile([128, 2, 512], BF16, name="tb", tag="tb")
                    for h in range(2):
                        nc.vector.tensor_scalar(
                            t[:, h, :],
                            QT[h][:],
                            KTn[h][:, j : j + 1],
                            0.0,
                            AL.add,
                            AL.max,
                        )
                        nc.tensor.matmul(
                            psp[32 * jj : 32 * jj + 32, half, :],
                            selb[h][:],
                            t[:, h, :],
                            start=(h == 0),
                            stop=(h == 1),
                            tile_position=(0, 32 * jj),
                            skip_group_check=True,
                        )

            def issue_s_units(q, half):
                """ScalarE-path j's: bf16 t via ACT, same 32-col matmuls,
                issued LAST in the quad so the PE never waits on ScalarE."""
                g = 2 * q + half
                psp = state[q]["ps"]
                s_jjs = _s_jjs(g)
                for jj in s_jjs:
                    j = 4 * g + jj
                    t = tpool.tile([128, 2, 512], BF16, name="ts", tag="ts")
                    for h in range(2):
                        nc.scalar.activation(
                            t[:, h, :],
                            QT[h][:],
                            AF.Relu,
                            bias=KTn[h][:, j : j + 1],
                            scale=1.0,
                        )
                        nc.tensor.matmul(
                            psp[32 * jj : 32 * jj + 32, half, :],
                            selb[h][:],
                            t[:, h, :],
                            start=(h == 0),
                            stop=(h == 1),
                            tile_position=(0, 32 * jj),
                            skip_group_check=True,
                        )

            def issue_exp(q, half=None):
                # p = exp(z/16 + b) over the pair's 2-bank PSUM span (or one
                # 512 half for the final drain)
                if "p" not in state[q]:
                    state[q]["p"] = wpool.tile(
                        [128, 2, L1], BF16, name="p", tag="p", bufs=3
                    )
                p = state[q]["p"]
                src = state[q]["ps"]
                if half is None:
                    nc.scalar.activation(
                        p[:], src[:], AF.Exp, bias=bml[:], scale=EXP_SCALE
                    )
                else:
                    nc.scalar.activation(
                        p[:, half, :],
                        src[:, half, :],
                        AF.Exp,
                        bias=bml[:],
                        scale=EXP_SCALE,
                    )

            def issue_pc(q, half=None):
                # pc = max(p, 1) = exp(relu(z/16 + b))
                if "pc" not in state[q]:
                    state[q]["pc"] = wpool.tile(
                        [128, 2, L1], BF16, name="pc", tag="pc", bufs=4
                    )
                pc = state[q]["pc"]
                p = state[q]["p"]
                if half is None:
                    nc.vector.tensor_scalar(pc[:], p[:], 1.0, None, AL.max)
                else:
                    nc.vector.tensor_scalar(
                        pc[:, half, :], p[:, half, :], 1.0, None, AL.max
                    )

            def issue_vmm_half(q, half, eh_outer=False):
                pc = state[q]["pc"]
                g = 2 * q + half
                ehs = [0, 1]
                for eh in ehs:
                    off = half * D + eh * 128
                    nc.tensor.matmul(
                        ops[eh][:],
                        V_sel[q][:, off : off + 128],
                        pc[:, half, :],
                        start=(g == 0),
                        stop=(g == NQUAD - 1),
                        skip_group_check=True,
                    )
                # denominator per quad half (PE writes can't cross PSUM banks)
                nc.tensor.matmul(
                    sps[:, half, :],
                    ones_sel[:, 0:N],
                    pc[:, half, :],
                    start=(q == 0),
                    stop=(q == NPAIR - 1),
                    skip_group_check=True,
                )

            def issue_vmm(q):
                issue_vmm_half(q, 0)
                issue_vmm_half(q, 1)
                del state[q]

            for q in range(NPAIR):
                if q + 2 < NPAIR:
                    build_pair(q + 2)
                state[q] = {
                    "ps": ps_pool.tile([128, 2, L1], F32, name="ps", tag="ps_s")
                }
                last = q == NPAIR - 1
                if q >= 1:
                    issue_exp(q - 1)
                issue_v_units(q, 0, (0, 1, 2))
                if q >= 1:
                    issue_pc(q - 1)
                issue_v_units(q, 0, (3,))
                issue_s_units(q, 0)
                if q >= 1:
                    # spread the previous pair's V-matmuls across the
                    # iteration so PE occupancy stays smooth (HAM never sees
                    # a >3.4us idle window mid-loop)
                    issue_vmm_half(q - 1, 0)
                if last:
                    issue_exp(q, 0)
                issue_v_units(q, 1, (0, 1, 2))
                if q >= 1:
                    issue_vmm_half(q - 1, 1)
                    del state[q - 1]
                if last:
                    issue_pc(q, 0)
                issue_v_units(q, 1, (3,))
                issue_s_units(q, 1)
                if last:
                    # drain the final pair per half so the output evacuation
                    # overlaps the second half's exp/pc chain
                    issue_vmm_half(q, 0)
                    issue_exp(q, 1)
                    issue_pc(q, 1)
                    issue_vmm_half(q, 1)

            # ---- evacuate + store ----
            for eh in range(2):
                ou = wpool.tile([128, L1], F32, name="ou", tag="ou", bufs=2)
                if eh == 0:
                    nc.vector.tensor_copy(ou[:], ops[eh][:])
                else:
                    nc.scalar.copy(ou[:], ops[eh][:])
                nc.sync.dma_start(outp_d[eh], ou[:])
            so = wpool.tile([16, 2, L1], F32, name="so")
            nc.scalar.copy(so[:], sps[:])
            nc.sync.dma_start(souts_d[:], so[:])

    nc.compile()
    return nc


_CACHE: dict = {}


def _get_graph(b_val: float):
    key = round(float(b_val), 10)
    if key not in _CACHE:
        _CACHE[key] = _build(float(b_val))
    return _CACHE[key]


def _host_prep(x_source, x_target, Wq, Wk, Wv, w_mlp):
    """Build per-core input maps (numpy, bf16/fp8)."""
    w_full = np.tile(np.asarray(w_mlp, np.float32), D // G)  # w_full[d] = w[d%16]
    # bf16 sel: [2 h][128 part, 32 cols], col = group(d); with the x8 in
    # Wq/Wk this emits z*16 (EXP_SCALE undoes it)
    selb = np.zeros((2, 128, 32), np.float32)
    for h in range(2):
        for dl in range(128):
            d = 128 * h + dl
            selb[h, dl, d // G] = SEL_SCALE * w_full[d]
    # V_sel mask: row p = 32*jj + s (s<16 valid), col e: keep if e%16 == s
    vmask = np.zeros((128, 2 * D), np.float32)
    for p in range(128):
        s = p % 32
        if s < 16:
            vmask[p, s::G] = 1.0
    # S selector: row p = 32*jj + s -> column s (s < 16)
    ones_sel = np.zeros((128, N), np.float32)
    for p in range(128):
        s = p % 32
        if s < 16:
            ones_sel[p, s] = 1.0

    def split_h(a):  # (256, X) -> (128, 2X): [dl, h*X+x] = a[128h+dl, x]
        X = a.shape[1]
        return np.ascontiguousarray(
            a.reshape(2, 128, X).transpose(1, 0, 2)
        ).reshape(128, 2 * X)

    wq_b = split_h(SCALE_T * np.asarray(Wq, np.float32).T).astype(BF)
    wk_b = split_h(SCALE_T * np.asarray(Wk, np.float32).T).astype(BF)
    wv_b = split_h(np.asarray(Wv, np.float32).T).astype(BF)
    selb_b = selb.astype(BF)
    vmask_b = vmask.astype(BF)
    ones_b = ones_sel.astype(BF)

    xtT = [
        split_h(np.asarray(x_target[b], np.float32).T).astype(BF)
        for b in range(B)
    ]
    xsT = [np.asarray(x_source[b], np.float32).T for b in range(B)]
    in_maps = []
    for core in range(NCORES):
        b, jq = divmod(core, 4)
        j0 = jq * JSH
        xssT = split_h(xsT[b][:, j0 : j0 + JSH]).astype(BF)
        in_maps.append(
            {
                "xtT": xtT[b],
                "xssT": xssT,
                "wqT": wq_b,
                "wkT": wk_b,
                "wvT": wv_b,
                "selb": selb_b,
                "vmask": vmask_b,
                "ones_sel": ones_b,
            }
        )
    return in_maps


def _host_gather(results):
    """Sum partials over j-shards, normalize, reshape to (B, L1, D)."""
    out = np.empty((B, L1, D), np.float32)
    for b in range(B):
        cores = [b * 4 + jq for jq in range(4)]
        U = sum(
            results[c]["outp"].reshape(D, L1).astype(np.float64) for c in cores
        )  # (e, i)
        S = sum(
            results[c]["souts"].sum(axis=1).astype(np.float64) for c in cores
        )  # (nn, i): paired denominator halves summed
        att = U / S[np.arange(D) % N, :]  # (e, i)
        out[b] = att.T.astype(np.float32)
    return out


def run(inputs, trace=False, **kwargs):
    nc = _get_graph(float(np.asarray(inputs["b_mlp"]).reshape(-1)[0]))
    in_maps = _host_prep(
        inputs["x_source"],
        inputs["x_target"],
        inputs["Wq"],
        inputs["Wk"],
        inputs["Wv"],
        inputs["w_mlp"],
    )
    res = run_bass_kernel_spmd(
        nc, in_maps, core_ids=list(range(NCORES)), trace=trace, **kwargs
    )
    return _host_gather(res.results), res


def kernel(**inputs) -> np.ndarray:
    out, _ = run(inputs, trace=False)
    return out
